# revision 8
# baseline (speedup 1.0000x reference)
"""Distributed MHA kernel for Trainium2 (8 NeuronCores, SPMD).

Problem: b=2, s=2048, e=2048, 32 heads x 64 dim, rotary_dim=32, causal,
fp32 reference.  Sharding: core c = batch*4 + head_group, i.e. each core
handles one batch and 8 heads (tensor-parallel over heads, data-parallel
over batch).  Column-parallel Wqkv, row-parallel Wout; the 4 partial
outputs per batch are summed on the host (cheap: 4 x 16.8 MB).

Per-core math (all matmuls in bf16, accumulation fp32 in PSUM):
  qkvT = Wqkv_g^T x^T           (f on partitions; q,k get RoPE via DVE)
  v    = x Wv_g                 (s on partitions, +ones column for denom)
  sT[u,t] = kT^T qT             (scoresT; exp on ACT, no max-sub needed:
                                 logits are O(5), fp32 exp is safe)
  oT[65,t] += v_ext^T pT        (row 64 accumulates the softmax denom)
  aT = oT * bcast(1/den)        (denominator broadcast via K=1 matmul)
  y[t,e] = aT^T Wout_g          (partial; host sums groups + bias)
"""

import numpy as np

S = 2048
E = 2048
NET = 16          # e-tiles of 128
SCH = 512         # s-chunk
NCH = 4           # s-chunks


def _build_nc():
    import concourse.bacc as bacc
    import concourse.bass as bass  # noqa: F401
    import concourse.tile as tile
    from concourse import mybir

    f32 = mybir.dt.float32
    bf16 = mybir.dt.bfloat16
    AF = mybir.ActivationFunctionType

    nc = bacc.Bacc(None, target_bir_lowering=False)
    xT = nc.dram_tensor("xT", [E, S], bf16, kind="ExternalInput")
    wqkv = nc.dram_tensor("wqkv", [E, 1536], bf16, kind="ExternalInput")
    wout = nc.dram_tensor("wout", [512, E], bf16, kind="ExternalInput")
    bqk = nc.dram_tensor("bqk", [128, 8], f32, kind="ExternalInput")
    bv = nc.dram_tensor("bv", [128, 512], f32, kind="ExternalInput")
    crep = nc.dram_tensor("crep", [128, S], f32, kind="ExternalInput")
    srep = nc.dram_tensor("srep", [128, S], f32, kind="ExternalInput")
    masks = nc.dram_tensor("masks", [4, 128, SCH], bf16, kind="ExternalInput")
    y = nc.dram_tensor("y", [S, E], f32, kind="ExternalOutput")

    with tile.TileContext(nc) as tc:
        from contextlib import ExitStack

        with ExitStack() as ctx:
            consts = ctx.enter_context(tc.tile_pool(name="consts", bufs=1))
            qkp = ctx.enter_context(tc.tile_pool(name="qkp", bufs=1))
            qjp = ctx.enter_context(tc.tile_pool(name="qjp", bufs=2))
            vp = ctx.enter_context(tc.tile_pool(name="vp", bufs=1))
            atp = ctx.enter_context(tc.tile_pool(name="atp", bufs=1))
            xp = ctx.enter_context(tc.tile_pool(name="xp", bufs=2))
            ptp = ctx.enter_context(tc.tile_pool(name="ptp", bufs=3))
            rtp = ctx.enter_context(tc.tile_pool(name="rtp", bufs=2))
            dnp = ctx.enter_context(tc.tile_pool(name="dnp", bufs=1))
            rbp = ctx.enter_context(tc.tile_pool(name="rbp", bufs=2))
            yp_sb = ctx.enter_context(tc.tile_pool(name="yp_sb", bufs=2))
            ps_a = ctx.enter_context(
                tc.tile_pool(name="ps_a", bufs=3, space="PSUM"))
            ps_o = ctx.enter_context(
                tc.tile_pool(name="ps_o", bufs=1, space="PSUM"))
            ps_m = ctx.enter_context(
                tc.tile_pool(name="ps_m", bufs=2, space="PSUM"))

            # ---- resident constants
            w_sb = []
            for et in range(NET):
                t = consts.tile([128, 1536], bf16, tag=f"w{et}")
                nc.sync.dma_start(t, wqkv[et * 128:(et + 1) * 128, :])
                w_sb.append(t)
            wo_sb = []
            for pr in range(4):
                t = consts.tile([128, E], bf16, tag=f"wo{pr}")
                nc.sync.dma_start(t, wout[pr * 128:(pr + 1) * 128, :])
                wo_sb.append(t)
            crep_sb = consts.tile([128, S], f32, tag="crep")
            nc.sync.dma_start(crep_sb, crep[:, :])
            srep_sb = consts.tile([128, S], f32, tag="srep")
            nc.sync.dma_start(srep_sb, srep[:, :])
            bqk_sb = consts.tile([128, 8], f32, tag="bqk")
            nc.sync.dma_start(bqk_sb, bqk[:, :])
            bv_sb = consts.tile([128, 512], f32, tag="bv")
            nc.sync.dma_start(bv_sb, bv[:, :])
            mask_sb = []
            for k in range(4):
                t = consts.tile([128, SCH], bf16, tag=f"mask{k}")
                nc.sync.dma_start(t, masks[k, :, :])
                mask_sb.append(t)
            ones_sb = consts.tile([1, 64], bf16, tag="ones")
            nc.vector.memset(ones_sb, 1.0)

            qk_t = {}
            v_t = {}
            at_t = {}

            for tj in range(NCH):
                # ======== phase A: project chunk tj ========
                xs = []
                for et in range(NET):
                    t = xp.tile([128, SCH], bf16, tag=f"x{et}")
                    nc.sync.dma_start(
                        t, xT[et * 128:(et + 1) * 128,
                              tj * SCH:(tj + 1) * SCH])
                    xs.append(t)
                for ft in range(8):  # q: 0..3, k: 4..7
                    ps = ps_a.tile([128, 512], f32, tag="a")
                    for et in range(NET):
                        nc.tensor.matmul(
                            ps,
                            lhsT=w_sb[et][:, ft * 128:(ft + 1) * 128],
                            rhs=xs[et],
                            start=(et == 0), stop=(et == NET - 1))
                    if ft < 4:
                        qt = qjp.tile([128, SCH], bf16, tag=f"q{ft}")
                    else:
                        qt = qkp.tile([128, SCH], bf16, tag=f"k{ft}_{tj}")
                    nc.vector.tensor_scalar_add(qt, ps, bqk_sb[:, ft:ft + 1])
                    # RoPE (both 64-row head halves of this f-tile).
                    # The half-rotation swap is a +-16-partition shift;
                    # engine APs need 32-aligned partition starts, so do
                    # the swap with SBUF->SBUF DMAs instead.
                    tmp = rtp.tile([128, SCH], bf16, tag="rtmp")
                    cs = slice(tj * SCH, (tj + 1) * SCH)
                    for hh in (0, 64):
                        nc.sync.dma_start(
                            tmp[hh:hh + 16, :], qt[hh + 16:hh + 32, :])
                        nc.sync.dma_start(
                            tmp[hh + 16:hh + 32, :], qt[hh:hh + 16, :])
                    for hh in (0, 64):
                        nc.vector.tensor_mul(
                            tmp[hh:hh + 32, :], tmp[hh:hh + 32, :],
                            srep_sb[hh:hh + 32, cs])
                        nc.vector.tensor_mul(
                            qt[hh:hh + 32, :], qt[hh:hh + 32, :],
                            crep_sb[hh:hh + 32, cs])
                        nc.vector.tensor_add(
                            qt[hh:hh + 32, :], qt[hh:hh + 32, :],
                            tmp[hh:hh + 32, :])
                    qk_t[(ft, tj)] = qt
                for us in range(4):
                    ut = tj * 4 + us
                    ps = ps_a.tile([128, 512], f32, tag="a")
                    for et in range(NET):
                        nc.tensor.matmul(
                            ps,
                            lhsT=xs[et][:, us * 128:(us + 1) * 128],
                            rhs=w_sb[et][:, 1024:1536],
                            start=(et == 0), stop=(et == NET - 1))
                    vt = vp.tile([128, 8, 65], bf16, tag=f"v{ut}")
                    nc.vector.tensor_add(
                        vt[:, :, 0:64],
                        ps.rearrange("p (h d) -> p h d", h=8),
                        bv_sb.rearrange("p (h d) -> p h d", h=8))
                    nc.vector.memset(vt[:, :, 64:65], 1.0)
                    v_t[ut] = vt

                # ======== phase B: attention rows t in chunk tj ========
                nu = 4 * tj + 4
                for pr in range(4):
                    oTs = []
                    for sub in range(2):
                        h = 2 * pr + sub
                        fq = h // 2
                        po = (h % 2) * 64
                        oT = ps_o.tile([65, 512], f32, tag=f"o{sub}")
                        for ut in range(nu):
                            jj, us = divmod(ut, 4)
                            st = ps_a.tile([128, 512], f32, tag="a")
                            nc.tensor.matmul(
                                st,
                                lhsT=qk_t[(4 + fq, jj)][po:po + 64,
                                                        us * 128:(us + 1) * 128],
                                rhs=qk_t[(fq, tj)][po:po + 64, :],
                                start=True, stop=True)
                            pt = ptp.tile([128, 512], bf16, tag="pt")
                            nc.scalar.activation(pt, st, AF.Exp, scale=0.125)
                            if ut >= 4 * tj:
                                nc.vector.tensor_mul(
                                    pt, pt, mask_sb[ut - 4 * tj])
                            nc.tensor.matmul(
                                oT, lhsT=v_t[ut][:, h, :], rhs=pt,
                                start=(ut == 0), stop=(ut == nu - 1))
                        oTs.append(oT)
                    # denominators -> reciprocal -> broadcast via K=1 matmul
                    rcbs = []
                    for sub in range(2):
                        dn = dnp.tile([1, 512], f32, tag=f"dn{sub}")
                        nc.vector.tensor_copy(dn, oTs[sub][64:65, :])
                        nc.vector.reciprocal(dn, dn)
                        rcb = dnp.tile([1, 512], bf16, tag=f"rcb{sub}")
                        nc.vector.tensor_copy(rcb, dn)
                        rcbs.append(rcb)
                    rb_ps = ps_m.tile([128, 512], f32, tag="m")
                    nc.tensor.matmul(rb_ps[0:64, :], lhsT=ones_sb,
                                     rhs=rcbs[0], start=True, stop=True)
                    nc.tensor.matmul(rb_ps[64:128, :], lhsT=ones_sb,
                                     rhs=rcbs[1], start=True, stop=True)
                    rb_sb = rbp.tile([128, 512], bf16, tag="rb")
                    nc.vector.tensor_copy(rb_sb, rb_ps)
                    at = atp.tile([128, 512], bf16, tag=f"at{pr}_{tj}")
                    nc.vector.tensor_mul(
                        at[0:64, :], oTs[0][0:64, :], rb_sb[0:64, :])
                    nc.vector.tensor_mul(
                        at[64:128, :], oTs[1][0:64, :], rb_sb[64:128, :])
                    at_t[(pr, tj)] = at

                # ======== phase C: output projection for chunk tj ========
                for ttl in range(4):
                    tt = tj * 4 + ttl
                    for ec in range(4):
                        yp = ps_m.tile([128, 512], f32, tag="m")
                        for pr in range(4):
                            nc.tensor.matmul(
                                yp,
                                lhsT=at_t[(pr, tj)][:, ttl * 128:(ttl + 1) * 128],
                                rhs=wo_sb[pr][:, ec * 512:(ec + 1) * 512],
                                start=(pr == 0), stop=(pr == 3))
                        ys = yp_sb.tile([128, 512], f32, tag="ys")
                        nc.vector.tensor_copy(ys, yp)
                        nc.sync.dma_start(
                            y[tt * 128:(tt + 1) * 128,
                              ec * 512:(ec + 1) * 512], ys)
    nc.compile()
    return nc


_CACHE = {}


def _host_consts():
    import ml_dtypes
    bf = ml_dtypes.bfloat16
    inv = 1.0 / (10000.0 ** (np.arange(0, 32, 2, dtype=np.float64) / 32.0))
    t = np.arange(S, dtype=np.float64)
    fr = np.outer(t, inv)                       # [s, 16]
    cos = np.cos(fr).astype(np.float32).T       # [16, s]
    sin = np.sin(fr).astype(np.float32).T
    crep = np.ones((128, S), np.float32)
    srep = np.zeros((128, S), np.float32)
    for blk in (0, 64):
        crep[blk:blk + 16] = cos
        crep[blk + 16:blk + 32] = cos
        srep[blk:blk + 16] = -sin
        srep[blk + 16:blk + 32] = sin
    m = np.zeros((4, 128, SCH), np.float32)
    ui = np.arange(128)[:, None]
    tcol = np.arange(SCH)[None, :]
    for k in range(4):
        m[k] = ((128 * k + ui) <= tcol).astype(np.float32)
    return crep, srep, m.astype(bf)


def kernel(**inputs):
    import ml_dtypes
    from concourse.bass_utils import run_bass_kernel_spmd

    x = np.asarray(inputs["x"], np.float32)
    Wqkv = np.asarray(inputs["Wqkv"], np.float32)
    bqkv = np.asarray(inputs["bqkv"], np.float32)
    Wout = np.asarray(inputs["Wout"], np.float32)
    bout = np.asarray(inputs["bout"], np.float32)

    if "nc" not in _CACHE:
        _CACHE["nc"] = _build_nc()
    nc = _CACHE["nc"]

    bf = ml_dtypes.bfloat16
    crep, srep, masks = _host_consts()
    in_maps = []
    for c in range(8):
        b, g = divmod(c, 4)
        gs = slice(g * 512, (g + 1) * 512)
        wq = Wqkv[:, 0:2048][:, gs]
        wk = Wqkv[:, 2048:4096][:, gs]
        wv = Wqkv[:, 4096:6144][:, gs]
        bq = bqkv[0:2048][gs]
        bk = bqkv[2048:4096][gs]
        bvv = bqkv[4096:6144][gs]
        in_maps.append(dict(
            xT=np.ascontiguousarray(x[b].T).astype(bf),
            wqkv=np.concatenate([wq, wk, wv], axis=1).astype(bf),
            wout=Wout[gs, :].astype(bf),
            bqk=np.concatenate([bq, bk]).reshape(8, 128).T.astype(
                np.float32).copy(),
            bv=np.broadcast_to(
                bvv.astype(np.float32), (128, 512)).copy(),
            crep=crep, srep=srep, masks=masks,
        ))
    kwargs = _CACHE.get("run_kwargs", {})
    res = run_bass_kernel_spmd(nc, in_maps, list(range(8)), **kwargs)
    _CACHE["last_results"] = res
    out = np.zeros((2, S, E), np.float32)
    for c in range(8):
        out[c // 4] += res.results[c]["y"]
    out += bout[None, None, :]
    return out


# revision 9
# speedup vs baseline: 1.2776x; 1.2776x over previous
"""Distributed MHA kernel for Trainium2 (8 NeuronCores, SPMD).

Problem: b=2, s=2048, e=2048, 32 heads x 64 dim, rotary_dim=32, causal,
fp32 reference.  Sharding: core c = batch*4 + head_group, i.e. each core
handles one batch and 8 heads (tensor-parallel over heads, data-parallel
over batch).  Column-parallel Wqkv, row-parallel Wout; the 4 partial
outputs per batch are summed on the host (cheap: 4 x 16.8 MB).

Per-core math (all matmuls in bf16, accumulation fp32 in PSUM):
  qkvT = Wqkv_g^T x^T           (f on partitions; q,k get RoPE via DVE)
  v    = x Wv_g                 (s on partitions, +ones column for denom)
  sT[u,t] = kT^T qT             (scoresT; exp on ACT, no max-sub needed:
                                 logits are O(5), fp32 exp is safe)
  oT[65,t] += v_ext^T pT        (row 64 accumulates the softmax denom)
  aT = oT * bcast(1/den)        (denominator broadcast via K=1 matmul)
  y[t,e] = aT^T Wout_g          (partial; host sums groups + bias)
"""

import numpy as np

S = 2048
E = 2048
NET = 16          # e-tiles of 128
SCH = 512         # s-chunk
NCH = 4           # s-chunks


def _build_nc():
    import concourse.bacc as bacc
    import concourse.bass as bass  # noqa: F401
    import concourse.tile as tile
    from concourse import mybir

    f32 = mybir.dt.float32
    bf16 = mybir.dt.bfloat16
    AF = mybir.ActivationFunctionType

    nc = bacc.Bacc(None, target_bir_lowering=False)
    xT = nc.dram_tensor("xT", [E, S], bf16, kind="ExternalInput")
    wqkv = nc.dram_tensor("wqkv", [E, 1536], bf16, kind="ExternalInput")
    wout = nc.dram_tensor("wout", [512, E], bf16, kind="ExternalInput")
    bqk = nc.dram_tensor("bqk", [128, 8], f32, kind="ExternalInput")
    bv = nc.dram_tensor("bv", [128, 512], f32, kind="ExternalInput")
    crep = nc.dram_tensor("crep", [128, S], bf16, kind="ExternalInput")
    srep = nc.dram_tensor("srep", [128, S], bf16, kind="ExternalInput")
    masks = nc.dram_tensor("masks", [4, 128, SCH], bf16, kind="ExternalInput")
    y = nc.dram_tensor("y", [S, E], f32, kind="ExternalOutput")

    with tile.TileContext(nc) as tc:
        from contextlib import ExitStack

        with ExitStack() as ctx:
            consts = ctx.enter_context(tc.tile_pool(name="consts", bufs=1))
            qkp = ctx.enter_context(tc.tile_pool(name="qkp", bufs=1))
            qjp = ctx.enter_context(tc.tile_pool(name="qjp", bufs=2))
            vp = ctx.enter_context(tc.tile_pool(name="vp", bufs=1))
            atp = ctx.enter_context(tc.tile_pool(name="atp", bufs=1))
            xp = ctx.enter_context(tc.tile_pool(name="xp", bufs=2))
            ptp = ctx.enter_context(tc.tile_pool(name="ptp", bufs=3))
            rtp = ctx.enter_context(tc.tile_pool(name="rtp", bufs=2))
            dnp = ctx.enter_context(tc.tile_pool(name="dnp", bufs=1))
            rbp = ctx.enter_context(tc.tile_pool(name="rbp", bufs=2))
            yp_sb = ctx.enter_context(tc.tile_pool(name="yp_sb", bufs=2))
            ps_a = ctx.enter_context(
                tc.tile_pool(name="ps_a", bufs=2, space="PSUM"))
            ps_s = ctx.enter_context(
                tc.tile_pool(name="ps_s", bufs=2, space="PSUM"))
            ps_o = ctx.enter_context(
                tc.tile_pool(name="ps_o", bufs=1, space="PSUM"))
            ps_m = ctx.enter_context(
                tc.tile_pool(name="ps_m", bufs=2, space="PSUM"))

            # ---- resident constants
            w_sb = []
            for et in range(NET):
                t = consts.tile([128, 1536], bf16, tag=f"w{et}")
                nc.sync.dma_start(t, wqkv[et * 128:(et + 1) * 128, :])
                w_sb.append(t)
            wo_sb = []
            for pr in range(4):
                t = consts.tile([128, E], bf16, tag=f"wo{pr}")
                nc.sync.dma_start(t, wout[pr * 128:(pr + 1) * 128, :])
                wo_sb.append(t)
            crep_sb = consts.tile([128, S], bf16, tag="crep")
            nc.sync.dma_start(crep_sb, crep[:, :])
            srep_sb = consts.tile([128, S], bf16, tag="srep")
            nc.sync.dma_start(srep_sb, srep[:, :])
            bqk_sb = consts.tile([128, 8], f32, tag="bqk")
            nc.sync.dma_start(bqk_sb, bqk[:, :])
            bv_sb = consts.tile([128, 512], f32, tag="bv")
            nc.sync.dma_start(bv_sb, bv[:, :])
            mask_sb = []
            for k in range(4):
                t = consts.tile([128, SCH], bf16, tag=f"mask{k}")
                nc.sync.dma_start(t, masks[k, :, :])
                mask_sb.append(t)
            ones_sb = consts.tile([1, 64], bf16, tag="ones")
            nc.vector.memset(ones_sb, 1.0)

            qk_t = {}
            v_t = {}
            at_t = {}

            for tj in range(NCH):
                # ======== phase A: project chunk tj ========
                xs = []
                for et in range(NET):
                    t = xp.tile([128, SCH], bf16, tag=f"x{et}")
                    nc.sync.dma_start(
                        t, xT[et * 128:(et + 1) * 128,
                              tj * SCH:(tj + 1) * SCH])
                    xs.append(t)
                for ft in range(8):  # q: 0..3, k: 4..7
                    ps = ps_a.tile([128, 512], f32, tag="a")
                    for et in range(NET):
                        nc.tensor.matmul(
                            ps,
                            lhsT=w_sb[et][:, ft * 128:(ft + 1) * 128],
                            rhs=xs[et],
                            start=(et == 0), stop=(et == NET - 1))
                    if ft < 4:
                        qt = qjp.tile([128, SCH], bf16, tag=f"q{ft}")
                    else:
                        qt = qkp.tile([128, SCH], bf16, tag=f"k{ft}_{tj}")
                    nc.vector.tensor_scalar_add(qt, ps, bqk_sb[:, ft:ft + 1])
                    # RoPE (both 64-row head halves of this f-tile).
                    # The half-rotation swap is a +-16-partition shift;
                    # engine APs need 32-aligned partition starts, so do
                    # the swap with SBUF->SBUF DMAs instead.
                    tmp = rtp.tile([128, SCH], bf16, tag="rtmp")
                    cs = slice(tj * SCH, (tj + 1) * SCH)
                    for hh in (0, 64):
                        nc.sync.dma_start(
                            tmp[hh:hh + 16, :], qt[hh + 16:hh + 32, :])
                        nc.sync.dma_start(
                            tmp[hh + 16:hh + 32, :], qt[hh:hh + 16, :])
                    nc.vector.tensor_mul(qt, qt, crep_sb[:, cs])
                    for hh in (0, 64):
                        nc.vector.tensor_mul(
                            tmp[hh:hh + 32, :], tmp[hh:hh + 32, :],
                            srep_sb[hh:hh + 32, cs])
                        nc.vector.tensor_add(
                            qt[hh:hh + 32, :], qt[hh:hh + 32, :],
                            tmp[hh:hh + 32, :])
                    qk_t[(ft, tj)] = qt
                for us in range(4):
                    ut = tj * 4 + us
                    ps = ps_a.tile([128, 512], f32, tag="a")
                    for et in range(NET):
                        nc.tensor.matmul(
                            ps,
                            lhsT=xs[et][:, us * 128:(us + 1) * 128],
                            rhs=w_sb[et][:, 1024:1536],
                            start=(et == 0), stop=(et == NET - 1))
                    vt = vp.tile([128, 8, 65], bf16, tag=f"v{ut}")
                    nc.vector.tensor_add(
                        vt[:, :, 0:64],
                        ps.rearrange("p (h d) -> p h d", h=8),
                        bv_sb.rearrange("p (h d) -> p h d", h=8))
                    nc.vector.memset(vt[:, :, 64:65], 1.0)
                    v_t[ut] = vt

                # ======== phase B: attention rows t in chunk tj ========
                nu = 4 * tj + 4
                for pr in range(4):
                    oTs = []
                    for sub in range(2):
                        h = 2 * pr + sub
                        fq = h // 2
                        po = (h % 2) * 64
                        oT = ps_o.tile([65, 512], f32, tag=f"o{sub}")
                        for ut in range(nu):
                            jj, us = divmod(ut, 4)
                            st = ps_s.tile([128, 512], f32, tag="s")
                            nc.tensor.matmul(
                                st,
                                lhsT=qk_t[(4 + fq, jj)][po:po + 64,
                                                        us * 128:(us + 1) * 128],
                                rhs=qk_t[(fq, tj)][po:po + 64, :],
                                start=True, stop=True)
                            pt = ptp.tile([128, 512], bf16, tag="pt")
                            nc.scalar.activation(pt, st, AF.Exp, scale=0.125)
                            if ut >= 4 * tj:
                                nc.vector.tensor_mul(
                                    pt, pt, mask_sb[ut - 4 * tj])
                            nc.tensor.matmul(
                                oT, lhsT=v_t[ut][:, h, :], rhs=pt,
                                start=(ut == 0), stop=(ut == nu - 1))
                        oTs.append(oT)
                    # denominators -> reciprocal -> broadcast via K=1 matmul
                    rcbs = []
                    for sub in range(2):
                        dn = dnp.tile([1, 512], f32, tag=f"dn{sub}")
                        nc.vector.tensor_copy(dn, oTs[sub][64:65, :])
                        rc = dnp.tile([1, 512], f32, tag=f"rc{sub}")
                        nc.vector.reciprocal_approx_fast(out=rc, in_=dn)
                        rcb = dnp.tile([1, 512], bf16, tag=f"rcb{sub}")
                        nc.vector.tensor_copy(rcb, rc)
                        rcbs.append(rcb)
                    rb_ps = ps_m.tile([128, 512], f32, tag="m")
                    nc.tensor.matmul(rb_ps[0:64, :], lhsT=ones_sb,
                                     rhs=rcbs[0], start=True, stop=True)
                    nc.tensor.matmul(rb_ps[64:128, :], lhsT=ones_sb,
                                     rhs=rcbs[1], start=True, stop=True)
                    rb_sb = rbp.tile([128, 512], bf16, tag="rb")
                    nc.vector.tensor_copy(rb_sb, rb_ps)
                    at = atp.tile([128, 512], bf16, tag=f"at{pr}_{tj}")
                    nc.vector.tensor_mul(
                        at[0:64, :], oTs[0][0:64, :], rb_sb[0:64, :])
                    nc.vector.tensor_mul(
                        at[64:128, :], oTs[1][0:64, :], rb_sb[64:128, :])
                    at_t[(pr, tj)] = at

                # ======== phase C: output projection for chunk tj ========
                for ttl in range(4):
                    tt = tj * 4 + ttl
                    for ec in range(4):
                        yp = ps_m.tile([128, 512], f32, tag="m")
                        for pr in range(4):
                            nc.tensor.matmul(
                                yp,
                                lhsT=at_t[(pr, tj)][:, ttl * 128:(ttl + 1) * 128],
                                rhs=wo_sb[pr][:, ec * 512:(ec + 1) * 512],
                                start=(pr == 0), stop=(pr == 3))
                        ys = yp_sb.tile([128, 512], f32, tag="ys")
                        nc.vector.tensor_copy(ys, yp)
                        nc.sync.dma_start(
                            y[tt * 128:(tt + 1) * 128,
                              ec * 512:(ec + 1) * 512], ys)
    nc.compile()
    return nc


_CACHE = {}


def _host_consts():
    import ml_dtypes
    bf = ml_dtypes.bfloat16
    inv = 1.0 / (10000.0 ** (np.arange(0, 32, 2, dtype=np.float64) / 32.0))
    t = np.arange(S, dtype=np.float64)
    fr = np.outer(t, inv)                       # [s, 16]
    cos = np.cos(fr).astype(np.float32).T       # [16, s]
    sin = np.sin(fr).astype(np.float32).T
    crep = np.ones((128, S), np.float32)
    srep = np.zeros((128, S), np.float32)
    for blk in (0, 64):
        crep[blk:blk + 16] = cos
        crep[blk + 16:blk + 32] = cos
        srep[blk:blk + 16] = -sin
        srep[blk + 16:blk + 32] = sin
    m = np.zeros((4, 128, SCH), np.float32)
    ui = np.arange(128)[:, None]
    tcol = np.arange(SCH)[None, :]
    for k in range(4):
        m[k] = ((128 * k + ui) <= tcol).astype(np.float32)
    return crep.astype(bf), srep.astype(bf), m.astype(bf)


def kernel(**inputs):
    import ml_dtypes
    from concourse.bass_utils import run_bass_kernel_spmd

    x = np.asarray(inputs["x"], np.float32)
    Wqkv = np.asarray(inputs["Wqkv"], np.float32)
    bqkv = np.asarray(inputs["bqkv"], np.float32)
    Wout = np.asarray(inputs["Wout"], np.float32)
    bout = np.asarray(inputs["bout"], np.float32)

    if "nc" not in _CACHE:
        _CACHE["nc"] = _build_nc()
    nc = _CACHE["nc"]

    bf = ml_dtypes.bfloat16
    crep, srep, masks = _host_consts()
    in_maps = []
    for c in range(8):
        b, g = divmod(c, 4)
        gs = slice(g * 512, (g + 1) * 512)
        wq = Wqkv[:, 0:2048][:, gs]
        wk = Wqkv[:, 2048:4096][:, gs]
        wv = Wqkv[:, 4096:6144][:, gs]
        bq = bqkv[0:2048][gs]
        bk = bqkv[2048:4096][gs]
        bvv = bqkv[4096:6144][gs]
        in_maps.append(dict(
            xT=np.ascontiguousarray(x[b].T).astype(bf),
            wqkv=np.concatenate([wq, wk, wv], axis=1).astype(bf),
            wout=Wout[gs, :].astype(bf),
            bqk=np.concatenate([bq, bk]).reshape(8, 128).T.astype(
                np.float32).copy(),
            bv=np.broadcast_to(
                bvv.astype(np.float32), (128, 512)).copy(),
            crep=crep, srep=srep, masks=masks,
        ))
    kwargs = _CACHE.get("run_kwargs", {})
    res = run_bass_kernel_spmd(nc, in_maps, list(range(8)), **kwargs)
    _CACHE["last_results"] = res
    out = np.zeros((2, S, E), np.float32)
    for c in range(8):
        out[c // 4] += res.results[c]["y"]
    out += bout[None, None, :]
    return out
